# revision 47
# baseline (speedup 1.0000x reference)
"""Trainium2 Bass kernel for naive causal MHA (dense transformer block).

Problem: x[2, 2048, 1024], per-head QKV (16 heads, head_dim 64), causal
softmax attention, concat heads, output projection.

Design (8 NeuronCores, tensor-parallel over heads):
  - core c computes QKV + attention for heads {2c, 2c+1}; AllToAlls (bf16)
    reshard y from head-split to token-split while compute continues; each
    core then computes the output projection for its 64-token sub-block of
    every q-chunk.
  - all matmul operands in bf16 (fp32r runs at half rate), fp32 accumulate.
  - the two heads' score matmuls are row-packed on the PE array (K=64 tiles
    at array rows 0-63 / 64-127 run concurrently) and both heads share one
    wide ACTIVATE (exp) across two psum banks.
  - causal handling: only t-blocks up to the diagonal are computed, and
    diagonal tiles slice scores/exp/AV to the valid column range; the
    128x128 diagonal sub-block is masked with one bf16 triangular multiply.
  - softmax: an extra ones-column in V accumulates the denominator in the
    AV matmul; one psum->sbuf copy releases the accumulator. Ordinary
    chunks broadcast the denominator row across partitions via a DRAM
    round trip; the FINAL chunk instead uses a K=1 PE matmul broadcast
    (ones64 x recip-row into psy's just-freed rows) with the den row
    copied on the idle ACT engine, so the last collective trigger fires
    ~6us after the final AV matmul. Custom-DVE recip runs at base 0 only.
  - the final q-chunk's y stores ride the (then idle) scalar hwdge queue
    so a busy sync queue can never delay the last collective trigger;
    the tail's yg loads split across both hwdge queues.
  - collective-gated work (yg loads, outproj matmuls) drains only into
    the last 8/4 t-block slots of a later chunk, by when its AllToAll is
    certainly done — drained earlier it head-of-line-blocks its whole
    engine queue (tile scheduler commits by modeled readiness, not
    emission order). x prefetches are scheduled off the first AllToAll's
    window to keep its duration stable.
  - a DVE-paced junk-matmul chain keeps the PE's HAM clock warm through
    the final collective wait (a >3.4us idle gap re-throttles the PE to
    half clock right before the last outproj), with a scheduler-order
    edge keeping the outproj matmuls behind it in the PE queue.
  - the in-order PE queue is kept dense by manually interleaving QKV and
    out-projection matmul groups into the gaps of the exp-paced attention
    pipeline (deque of work units drained per t-block).
"""

import contextlib
import ctypes
import math
import sys
import types
from collections import deque

import numpy as np
import ml_dtypes

import concourse.bacc as bacc
import concourse.mybir as mybir
import concourse.tile as tile
from concourse.bass import ds, _add_dep_helper as _add_dep

N_CORES = 8
B = 2
S = 2048
D = 1024
HD = 64

F32 = mybir.dt.float32
BF = mybir.dt.bfloat16

SC = 512          # q-chunk width
N_SC = S // SC    # 4
N_DC = D // 128   # 8 contraction chunks
N_TB = S // 128   # 16 t-blocks per batch


def _build_program(dbg=False):
    nc = bacc.Bacc(
        "TRN2", target_bir_lowering=False, debug=False, num_devices=N_CORES
    )

    xt_d = nc.dram_tensor("xt", [B, D, S], BF, kind="ExternalInput").ap()
    wq_d = nc.dram_tensor("wq", [D, 128], BF, kind="ExternalInput").ap()
    wk_d = nc.dram_tensor("wk", [D, 128], BF, kind="ExternalInput").ap()
    wv_d = nc.dram_tensor("wv", [D, 128], BF, kind="ExternalInput").ap()
    bq_d = nc.dram_tensor("bq", [128, 1], F32, kind="ExternalInput").ap()
    bk_d = nc.dram_tensor("bk", [128, 1], F32, kind="ExternalInput").ap()
    bv_d = nc.dram_tensor("bv", [1, 128], F32, kind="ExternalInput").ap()
    wout_d = nc.dram_tensor("wout", [D, D], BF, kind="ExternalInput").ap()
    bout_d = nc.dram_tensor("bout", [1, D], F32, kind="ExternalInput").ap()
    out_d = nc.dram_tensor("out", [512, D], F32, kind="ExternalOutput").ap()

    # per-(batch, half) resharding buffers; one AllToAll per pair of
    # q-chunks (small collectives are floor-bound ~10us on this topology,
    # so fewer/bigger wins)
    yp = {
        (b, h): nc.dram_tensor(f"yp{b}_{h}", [8, 2, 128, 64], BF)
        for b in range(B) for h in range(2)
    }
    ya = {
        (b, h): nc.dram_tensor(f"ya{b}_{h}", [8, 2, 128, 64], BF)
        for b in range(B) for h in range(2)
    }
    if dbg:
        dbg_qT = nc.dram_tensor("dbg_qT", [B, 128, S], BF, kind="ExternalOutput").ap()
        dbg_kT = nc.dram_tensor("dbg_kT", [B, 128, S], BF, kind="ExternalOutput").ap()
        dbg_v = nc.dram_tensor("dbg_v", [B, 128, N_TB * 2 * 66], BF, kind="ExternalOutput").ap()

    tri_d = nc.dram_tensor("tri", [128, 128], BF, kind="ExternalInput").ap()
    ones_d = nc.dram_tensor("vones", [128, N_TB, 2, 2], BF, kind="ExternalInput").ap()
    ones64_d = nc.dram_tensor("ones64", [1, 64], F32, kind="ExternalInput").ap()

    with tile.TileContext(nc) as tc, contextlib.ExitStack() as ctx:
        const = ctx.enter_context(tc.tile_pool(name="const", bufs=1))
        xt_pool = ctx.enter_context(tc.tile_pool(name="xtp", bufs=4))
        qk_pool = ctx.enter_context(tc.tile_pool(name="qkp", bufs=2))
        v_pool = ctx.enter_context(tc.tile_pool(name="vp", bufs=2))
        ex_pool = ctx.enter_context(tc.tile_pool(name="exp", bufs=4))
        z_pool = ctx.enter_context(tc.tile_pool(name="zp", bufs=2))
        y_pool = ctx.enter_context(tc.tile_pool(name="yp", bufs=3))
        yg_pool = ctx.enter_context(tc.tile_pool(name="ygp", bufs=2))
        o_pool = ctx.enter_context(tc.tile_pool(name="op", bufs=2))
        psum = ctx.enter_context(tc.tile_pool(name="psum", bufs=1, space="PSUM"))
        dram_pool = ctx.enter_context(tc.tile_pool(name="dramp", bufs=2, space="DRAM"))

        # ---- constant tiles ----
        wq_sb = const.tile([128, N_DC, 128], BF)
        wk_sb = const.tile([128, N_DC, 128], BF)
        wv_sb = const.tile([128, N_DC, 128], BF)
        bq_sb = const.tile([128, 1], F32)
        bk_sb = const.tile([128, 1], F32)
        bv_bc = const.tile([128, 128], F32)
        tri_sb = const.tile([128, 128], BF)
        ones64_sb = const.tile([1, 64], F32)
        warm = const.tile([128, 1], F32)

        # ---- per-batch state ----
        qTs, kTs, vsbs = {}, {}, {}

        def emit_xt_dma(b, sc, split=False):
            xt = xt_pool.tile([128, N_DC, SC], BF, tag="xt")
            src = xt_d[b].rearrange("(c p) s -> p c s", p=128)[:, :, ds(sc * SC, SC)]
            if split:
                for dc in range(N_DC):
                    nc.sync.dma_start(out=xt[:, dc, :], in_=src[:, dc, :])
            else:
                nc.sync.dma_start(out=xt, in_=src)
            return xt

        def emit_batch_tiles(b, q=None):
            q = q or nc.sync
            qT = qk_pool.tile([128, S], BF, tag="qT")
            kT = qk_pool.tile([128, S], BF, tag="kT")
            v_sb = v_pool.tile([128, N_TB, 2, 66], BF, tag="vsb")
            q.dma_start(out=v_sb[:, :, :, 64:66], in_=ones_d)
            qTs[b], kTs[b], vsbs[b] = qT, kT, v_sb

        def unit_q(b, sc, xt):
            ps = psum.tile([128, SC], F32, tag="qkv", bufs=2)
            for dc in range(N_DC):
                nc.tensor.matmul(
                    ps, wq_sb[:, dc, :], xt[:, dc, :],
                    start=(dc == 0), stop=(dc == N_DC - 1),
                )
            nc.vector.tensor_scalar_add(
                out=qTs[b][:, ds(sc * SC, SC)], in0=ps, scalar1=bq_sb
            )

        def unit_k(b, sc, xt):
            ps = psum.tile([128, SC], F32, tag="qkv", bufs=2)
            for dc in range(N_DC):
                nc.tensor.matmul(
                    ps, wk_sb[:, dc, :], xt[:, dc, :],
                    start=(dc == 0), stop=(dc == N_DC - 1),
                )
            nc.vector.tensor_scalar_add(
                out=kTs[b][:, ds(sc * SC, SC)], in0=ps, scalar1=bk_sb
            )

        def unit_v_j(b, sc, xt, j4, holder):
            # one t-sub-block of the V projection; finer grain so drained
            # units never dam up the attention pipeline
            if j4 == 0:
                holder["ps"] = psum.tile(
                    [128, SC], F32, tag="qkv", bufs=2, name="psv"
                )
            ps = holder["ps"]
            for dc in range(N_DC):
                nc.tensor.matmul(
                    ps[:, ds(j4 * 128, 128)],
                    xt[:, dc, ds(j4 * 128, 128)],
                    wv_sb[:, dc, :],
                    start=(dc == 0), stop=(dc == N_DC - 1),
                )
            nc.vector.tensor_add(
                out=vsbs[b][:, sc * 4 + j4, :, 0:64],
                in0=ps[:, ds(j4 * 128, 128)].rearrange("p (h e) -> p h e", h=2),
                in1=bv_bc.rearrange("p (h e) -> p h e", h=2),
            )

        def qkv_units(b, sc, xt):
            holder = {}
            us = [
                lambda: unit_q(b, sc, xt),
                lambda: unit_k(b, sc, xt),
            ]
            for j4 in range(4):
                us.append(lambda j4=j4: unit_v_j(b, sc, xt, j4, holder))
            return us

        # big constants, loaded in slices during the b0 q-chunk windows
        wout_sb = const.tile([128, N_DC, D], BF)
        bout_bc = const.tile([128, D], F32)
        wout_src = wout_d.rearrange("(c p) e -> p c e", p=128)

        def emit_wout_slices(lo, hi):
            for ec in range(lo, hi):
                nc.sync.dma_start(out=wout_sb[:, ec, :], in_=wout_src[:, ec, :])

        ygs = {}

        def unit_yg_load(b, rb, queues=None):
            # rows [128rb, 128rb+128) of this core's 256 tokens = chunks
            # {2rb, 2rb+1}; collectives for those chunks must be done
            queues = queues or [nc.sync]
            ygl = []
            for ec in range(8):
                yg = yg_pool.tile([128, 2, 64], BF, tag=f"yg{ec}")
                queues[ec % len(queues)].dma_start(
                    out=yg, in_=ya[(b, rb)].ap()[ec].transpose([1, 0, 2])
                )
                ygl.append(yg)
            ygs[(b, rb)] = ygl

        def unit_outproj(b, rb, ch, store_q=None, after=None):
            store_q = store_q or nc.sync
            pso = psum.tile([128, SC], F32, tag="qkv", bufs=2)
            for ec in range(8):
                mm = nc.tensor.matmul(
                    pso,
                    ygs[(b, rb)][ec].rearrange("p a i -> p (a i)"),
                    wout_sb[:, ec, ds(ch * SC, SC)],
                    start=(ec == 0), stop=(ec == 7),
                )
                if ec == 0 and after is not None:
                    # scheduler-order-only edge: keep these matmuls behind
                    # the PE warm-keeper chain in the queue
                    _add_dep(mm.ins, after.ins, sync=False,
                             reason="after warm chain")
            ot = o_pool.tile([128, SC], F32, tag="ot")
            nc.vector.tensor_add(out=ot, in0=pso, in1=bout_bc[:, ds(ch * SC, SC)])
            store_q.dma_start(
                out=out_d[ds(b * 256 + rb * 128, 128), ds(ch * SC, SC)], in_=ot
            )

        def outproj_units(b, rb):
            us = [lambda: unit_yg_load(b, rb)]
            for ch in range(2):
                us.append(lambda ch=ch: unit_outproj(b, rb, ch))
            return us

        def attn(b, qc, pending, mid=(), late=()):
            """Attention q-chunk qc for batch b; drains `pending` PE work
            units into the gaps of the exp-paced tb pipeline. `mid`/`late`
            units gate on collectives, so they only drain into the last
            8/4 slots (by when their AllToAll is certainly done — a unit
            drained too early head-of-line-blocks its whole engine queue)."""
            qT, kT, v_sb = qTs[b], kTs[b], vsbs[b]
            ntb = 4 * qc + 4
            last = b == 1 and qc == N_SC - 1
            psy = psum.tile([65, 2, SC], F32, tag="psy", bufs=1)
            mid, late = deque(mid), deque(late)
            for tb in range(ntb):
                j = tb - 4 * qc
                lo = 128 * j if j > 0 else 0
                w = SC - lo
                pss = psum.tile([128, 2, SC], F32, tag="pss", bufs=2)
                for h in range(2):
                    nc.tensor.matmul(
                        pss[:, h, ds(lo, w)],
                        kT[ds(64 * h, 64), ds(tb * 128, 128)],
                        qT[ds(64 * h, 64), ds(qc * SC + lo, w)],
                        start=True, stop=True,
                    )
                ex = ex_pool.tile([128, 2, SC], BF, tag="ex", bufs=4)
                nc.scalar.activation(
                    out=ex[:, :, ds(lo, w)], in_=pss[:, :, ds(lo, w)],
                    func=mybir.ActivationFunctionType.Exp,
                    scale=0.125,
                )
                if j >= 0:
                    for h in range(2):
                        nc.vector.tensor_mul(
                            out=ex[:, h, ds(128 * j, 128)],
                            in0=ex[:, h, ds(128 * j, 128)],
                            in1=tri_sb,
                        )
                # drain pending PE units into this slot (before the AV
                # matmuls, so they fill the PE while exp/psy-release settles)
                nleft = ntb - tb
                k = (len(pending) + nleft - 1) // nleft
                for _ in range(min(k, len(pending))):
                    pending.popleft()()
                if nleft <= 8 and mid:
                    k = (len(mid) + nleft - 1) // nleft
                    for _ in range(min(k, len(mid))):
                        mid.popleft()()
                if nleft <= 4 and late:
                    k = (len(late) + nleft - 1) // nleft
                    for _ in range(min(k, len(late))):
                        late.popleft()()
                for h in range(2):
                    nc.tensor.matmul(
                        psy[:, h, ds(lo, w)],
                        v_sb[:, tb, h, 0:65],
                        ex[:, h, ds(lo, w)],
                        start=(tb == 0), stop=(tb == ntb - 1),
                    )
            while pending:
                pending.popleft()()
            while mid:
                mid.popleft()()
            while late:
                late.popleft()()
            # normalization. The final q-chunk broadcasts the denominator
            # across partitions with a K=1 PE matmul into psy's own
            # (just-freed) rows 0-63 — no DRAM round trip, so the last
            # collective trigger fires ~AV+6us, and its stores ride the
            # (then idle) scalar queue. Earlier chunks, whose collectives
            # have many microseconds of slack, keep the cheaper DRAM
            # round-trip broadcast that stays off the PE. Custom-DVE recip
            # runs at base 0 only (it misreads psum/base!=0 operands).
            qn = nc.scalar if last else nc.sync
            if last:
                with tc.high_priority():
                    yu = z_pool.tile([64, 2, SC], F32, tag="yu", bufs=2)
                    nc.vector.tensor_copy(out=yu, in_=psy[0:64])
                    # den row copied on the (now idle) ACT engine so it
                    # overlaps the DVE's yu copy
                    d0 = z_pool.tile([1, 2 * SC], F32, tag="d0", bufs=2)
                    nc.scalar.activation(
                        out=d0,
                        in_=psy[ds(64, 1), :, :].rearrange("p a s -> p (a s)"),
                        func=mybir.ActivationFunctionType.Copy,
                    )
                    zi0 = z_pool.tile([1, 2 * SC], F32, tag="zi0", bufs=2)
                    nc.vector.reciprocal_approx_fast(out=zi0, in_=d0)
                    for h in range(2):
                        nc.tensor.matmul(
                            psy[0:64, h, :], ones64_sb,
                            zi0[:, ds(h * SC, SC)], start=True, stop=True,
                        )
                zi = psy[0:64]
            else:
                yu65 = z_pool.tile([65, 2, SC], F32, tag="yu65", bufs=2)
                nc.vector.tensor_copy(out=yu65, in_=psy)
                zd = dram_pool.tile([1, 2 * SC], F32, tag="zd", bufs=2)
                qn.dma_start(
                    out=zd, in_=yu65[ds(64, 1), :, :].rearrange("p a s -> p (a s)")
                )
                denb = z_pool.tile([64, 2 * SC], F32, tag="zb", bufs=2)
                qn.dma_start(out=denb, in_=zd.to_broadcast([64, 2 * SC]))
                ziw = z_pool.tile([64, 2, SC], F32, tag="zi", bufs=2)
                nc.vector.reciprocal_approx_fast(
                    out=ziw.rearrange("p a s -> p (a s)"), in_=denb
                )
                yu, zi = yu65, ziw
            hp = tc.high_priority() if last else contextlib.nullcontext()
            with hp:
                for h in range(2):
                    yts = y_pool.tile([64, 2, 256], BF, tag="yts", bufs=3)
                    nc.vector.tensor_mul(
                        out=yts.rearrange("p a s -> p (a s)"),
                        in0=yu[0:64, h, :], in1=zi[:, h, :],
                    )
                    qn.dma_start(
                        out=yp[(b, qc // 2)].ap()[:, qc % 2, ds(64 * h, 64), :]
                        .transpose([1, 0, 2]),
                        in_=yts.rearrange("p a s -> p (a s)").rearrange(
                            "p (j i) -> p j i", i=64
                        ),
                    )
            return psy

        def emit_collective(key):
            nc.gpsimd.collective_compute(
                "AllToAll",
                mybir.AluOpType.bypass,
                replica_groups=[list(range(N_CORES))],
                ins=[yp[key].ap()],
                outs=[ya[key].ap()],
            )

        # ================== main schedule ==================
        # startup-critical order: Q-unit deps first (wq, bq, x chunk 0 in
        # per-dc pieces), then the rest of the constants between them.
        xts = {}
        nc.sync.dma_start(out=wq_sb, in_=wq_d.rearrange("(c p) e -> p c e", p=128))
        nc.sync.dma_start(out=bq_sb, in_=bq_d)
        xts[(0, 0)] = emit_xt_dma(0, 0, split=True)
        nc.sync.dma_start(out=wk_sb, in_=wk_d.rearrange("(c p) e -> p c e", p=128))
        nc.sync.dma_start(out=bk_sb, in_=bk_d)
        nc.sync.dma_start(out=wv_sb, in_=wv_d.rearrange("(c p) e -> p c e", p=128))
        nc.sync.dma_start(out=bv_bc, in_=bv_d.to_broadcast([128, 128]))
        emit_batch_tiles(0)
        nc.sync.dma_start(out=tri_sb, in_=tri_d)
        nc.sync.dma_start(out=ones64_sb, in_=ones64_d)
        # preload the exp table set off the critical path
        nc.scalar.activation(
            out=warm, in_=bq_sb, func=mybir.ActivationFunctionType.Exp
        )
        xts[(0, 1)] = emit_xt_dma(0, 1, split=True)
        for u in qkv_units(0, 0, xts[(0, 0)]):
            u()

        # x prefetches are scheduled so no transfer overlaps the first
        # AllToAll's window (its duration is contention-sensitive and its
        # completion gates outproj(0,0))
        prefetch = {
            (0, 0): [(0, 2), (0, 3)],
            (0, 1): [(1, 0)],
            (0, 2): [(1, 1)],
            (1, 0): [(1, 3)],
        }
        # xt(1,2) is needed right at b1q1's start and must be fully landed
        # BEFORE the first AllToAll's window (~84-108us) or its transfer
        # both slows that collective and arrives too late for the drained
        # QKV units; its pool slot (xt(0,2)'s) is free from b0q1 on.
        prefetch_pre = {(0, 3): [(1, 2)]}
        for b in range(B):
            for qc in range(N_SC):
                for key in prefetch_pre.get((b, qc), []):
                    xts[key] = emit_xt_dma(*key)
                pending, mid, late = deque(), deque(), deque()
                if qc < 3:
                    pending.extend(qkv_units(b, qc + 1, xts[(b, qc + 1)]))
                elif b == 0:
                    # tail of b0: b1's first QKV
                    emit_batch_tiles(1)
                    pending.extend(qkv_units(1, 0, xts[(1, 0)]))
                if b == 1 and qc == 2:
                    pending.appendleft(lambda: unit_yg_load(0, 0))
                    mid.extend(
                        lambda ch=ch: unit_outproj(0, 0, ch) for ch in range(2)
                    )
                elif b == 1 and qc == 3:
                    pending.appendleft(lambda: unit_yg_load(0, 1))
                    mid.extend(
                        lambda ch=ch: unit_outproj(0, 1, ch) for ch in range(2)
                    )
                    late.append(lambda: unit_yg_load(1, 0))
                    late.extend(
                        lambda ch=ch: unit_outproj(1, 0, ch) for ch in range(2)
                    )
                psy_last = attn(b, qc, pending, mid, late)
                if qc % 2 == 1:
                    emit_collective((b, qc // 2))
                # prefetch + big-constant DMAs go AFTER the normalize so
                # the sync queue serves the collective trigger path first;
                # the data is still 1+ chunks early
                for key in prefetch.get((b, qc), []):
                    xts[key] = emit_xt_dma(*key)
                if b == 0 and qc == 0:
                    emit_wout_slices(0, 4)
                elif b == 0 and qc == 1:
                    emit_wout_slices(4, 8)
                    nc.sync.dma_start(out=bout_bc, in_=bout_d.to_broadcast([128, D]))

        # tail: yg loads for the final pair ride both hwdge queues; a
        # DVE-paced junk-matmul chain keeps the PE's HAM clock warm
        # through the collective wait (a >3.4us idle gap would re-throttle
        # the PE to half clock right before the final outproj).
        unit_yg_load(1, 1, queues=[nc.sync, nc.scalar])
        ja = z_pool.tile([128, 2048], BF, tag="junk", bufs=2)
        nc.vector.tensor_copy(
            out=ja, in_=wout_sb[:, 0:2, :].rearrange("p c e -> p (c e)")
        )
        last_junk = None
        for _ in range(18):
            last_junk = nc.tensor.matmul(
                psy_last[0:64, 0, :], wq_sb[:, 0, 0:64], ja[:, 0:SC],
                start=True, stop=True,
            )
            jb = z_pool.tile([128, 2048], BF, tag="junk", bufs=2)
            nc.vector.tensor_copy(out=jb, in_=ja)
            ja = jb
        unit_outproj(1, 1, 0, store_q=nc.sync, after=last_junk)
        unit_outproj(1, 1, 1, store_q=nc.scalar)

        if dbg:
            for b in range(B):
                nc.sync.dma_start(out=dbg_qT[b], in_=qTs[b])
                nc.sync.dma_start(out=dbg_kT[b], in_=kTs[b])
                nc.sync.dma_start(
                    out=dbg_v[b], in_=vsbs[b].rearrange("p a b c -> p (a b c)")
                )

    nc.compile()
    return nc


_NC_CACHE = {}


def _get_program(dbg=False):
    if dbg not in _NC_CACHE:
        _NC_CACHE[dbg] = _build_program(dbg=dbg)
    return _NC_CACHE[dbg]


def make_in_maps(x, Wqkv, bqkv, Wout, bout):
    x = np.asarray(x, dtype=np.float32)
    Wqkv = np.asarray(Wqkv, dtype=np.float32)
    bqkv = np.asarray(bqkv, dtype=np.float32)
    Wout = np.asarray(Wout, dtype=np.float32)
    bout = np.asarray(bout, dtype=np.float32)
    bf = lambda a: np.ascontiguousarray(a).astype(ml_dtypes.bfloat16)

    xt = bf(x.transpose(0, 2, 1))  # [B, D, S]
    wout = bf(Wout)
    bout2 = np.ascontiguousarray(bout.reshape(1, D))

    tri = np.triu(np.ones((128, 128), dtype=np.float32)).astype(ml_dtypes.bfloat16)
    vones = np.ones((128, N_TB, 2, 2), dtype=np.float32).astype(ml_dtypes.bfloat16)
    in_maps = []
    for c in range(N_CORES):
        h0, h1 = 2 * c, 2 * c + 1
        wq = bf(np.concatenate([Wqkv[h0, :, 0:64], Wqkv[h1, :, 0:64]], axis=1))
        wk = bf(np.concatenate([Wqkv[h0, :, 64:128], Wqkv[h1, :, 64:128]], axis=1))
        wv = bf(np.concatenate([Wqkv[h0, :, 128:192], Wqkv[h1, :, 128:192]], axis=1))
        bq = np.ascontiguousarray(
            np.concatenate([bqkv[h0, 0:64], bqkv[h1, 0:64]]).reshape(128, 1)
        )
        bk = np.ascontiguousarray(
            np.concatenate([bqkv[h0, 64:128], bqkv[h1, 64:128]]).reshape(128, 1)
        )
        bv = np.ascontiguousarray(
            np.concatenate([bqkv[h0, 128:192], bqkv[h1, 128:192]]).reshape(1, 128)
        )
        in_maps.append(
            {
                "xt": xt, "wq": wq, "wk": wk, "wv": wv,
                "bq": bq, "bk": bk, "bv": bv,
                "wout": wout, "bout": bout2,
                "tri": tri, "vones": vones,
                "ones64": np.ones((1, 64), dtype=np.float32),
            }
        )
    return in_maps


def assemble(results):
    # core c's rows: row b*256 + qc*64 + i  ->  token qc*512 + 64c + i
    full = np.empty((B, S, D), dtype=np.float32)
    for c in range(N_CORES):
        rc = results[c]["out"].reshape(B, N_SC, 64, D)
        for b in range(B):
            for qc in range(N_SC):
                full[b, qc * 512 + 64 * c : qc * 512 + 64 * (c + 1)] = rc[b, qc]
    return full


def _install_ntff_hook():
    if "antenv.axon_hooks" in sys.modules:
        return
    so_path = "/opt/axon/libaxon_pjrt.so"
    try:
        lib = ctypes.CDLL(so_path)
        lib.axon_start_nrt_profile.argtypes = [
            ctypes.POINTER(ctypes.c_int64),
            ctypes.c_size_t,
        ]
        lib.axon_start_nrt_profile.restype = ctypes.c_int64
        lib.axon_stop_nrt_profile.argtypes = [ctypes.c_char_p]
        lib.axon_stop_nrt_profile.restype = ctypes.c_int64
    except (OSError, AttributeError):
        return

    @contextlib.contextmanager
    def _hook(output_dir, device_ids):
        import jax

        jax.devices()
        if device_ids:
            ids = (ctypes.c_int64 * len(device_ids))(*device_ids)
            rc = lib.axon_start_nrt_profile(ids, len(device_ids))
        else:
            rc = lib.axon_start_nrt_profile(None, 0)
        if rc != 0:
            raise RuntimeError(f"axon_start_nrt_profile rc={rc}")
        try:
            yield
        finally:
            n = lib.axon_stop_nrt_profile(str(output_dir).encode())
            if n < 0:
                raise RuntimeError(f"axon_stop_nrt_profile rc={n}")

    mod = types.ModuleType("antenv.axon_hooks")
    mod.get_axon_ntff_profile_hook = lambda: _hook
    mod.set_axon_ntff_profile_hook = lambda h: None
    sys.modules["antenv.axon_hooks"] = mod


def run(inputs, trace=False, dbg=False):
    from concourse.bass_utils import run_bass_kernel_spmd

    if trace:
        _install_ntff_hook()
    nc = _get_program(dbg=dbg)
    in_maps = make_in_maps(**inputs)
    res = run_bass_kernel_spmd(
        nc, in_maps, core_ids=list(range(N_CORES)), trace=trace
    )
    return assemble(res.results), res


def kernel(x, Wqkv, bqkv, Wout, bout):
    out, _ = run(
        {"x": x, "Wqkv": Wqkv, "bqkv": bqkv, "Wout": Wout, "bout": bout},
        trace=False,
    )
    return out


# revision 52
# speedup vs baseline: 1.1172x; 1.1172x over previous
"""Trainium2 Bass kernel for naive causal MHA (dense transformer block).

Problem: x[2, 2048, 1024], per-head QKV (16 heads, head_dim 64), causal
softmax attention, concat heads, output projection.

Design (8 NeuronCores, tensor-parallel over heads):
  - core c computes QKV + attention for heads {2c, 2c+1}; AllToAlls (bf16)
    reshard y from head-split to token-split while compute continues; each
    core then computes the output projection for its 64-token sub-block of
    every q-chunk.
  - all matmul operands in bf16 (fp32r runs at half rate), fp32 accumulate.
  - the two heads' score matmuls are row-packed on the PE array (K=64 tiles
    at array rows 0-63 / 64-127 run concurrently) and both heads share one
    wide ACTIVATE (exp) across two psum banks.
  - causal handling: only t-blocks up to the diagonal are computed, and
    diagonal tiles slice scores/exp/AV to the valid column range; the
    128x128 diagonal sub-block is masked with one bf16 triangular multiply.
  - softmax: an extra ones-column in V accumulates the denominator in the
    AV matmul; one psum->sbuf copy releases the accumulator. Ordinary
    chunks broadcast the denominator row across partitions via a DRAM
    round trip; the FINAL chunk instead uses a K=1 PE matmul broadcast
    (ones64 x recip-row into psy's just-freed rows) with the den row
    copied on the idle ACT engine, so the last collective trigger fires
    ~6us after the final AV matmul. Custom-DVE recip runs at base 0 only.
  - the final q-chunk's y stores ride the (then idle) scalar hwdge queue
    so a busy sync queue can never delay the last collective trigger;
    the tail's yg loads split across both hwdge queues.
  - collective-gated work (yg loads, outproj matmuls) drains only into
    the last 8/4 t-block slots of a later chunk, by when its AllToAll is
    certainly done — drained earlier it head-of-line-blocks its whole
    engine queue (tile scheduler commits by modeled readiness, not
    emission order). x prefetches are scheduled off the first AllToAll's
    window to keep its duration stable.
  - a DVE-paced junk-matmul chain keeps the PE's HAM clock warm through
    the final collective wait (a >3.4us idle gap re-throttles the PE to
    half clock right before the last outproj), with a scheduler-order
    edge keeping the outproj matmuls behind it in the PE queue.
  - the in-order PE queue is kept dense by manually interleaving QKV and
    out-projection matmul groups into the gaps of the exp-paced attention
    pipeline (deque of work units drained per t-block).
"""

import contextlib
import ctypes
import math
import sys
import types
from collections import deque

import numpy as np
import ml_dtypes

import concourse.bacc as bacc
import concourse.mybir as mybir
import concourse.tile as tile
from concourse.bass import ds, _add_dep_helper as _add_dep

N_CORES = 8
B = 2
S = 2048
D = 1024
HD = 64

F32 = mybir.dt.float32
BF = mybir.dt.bfloat16

SC = 512          # q-chunk width
N_SC = S // SC    # 4
N_DC = D // 128   # 8 contraction chunks
N_TB = S // 128   # 16 t-blocks per batch


def _build_program(dbg=False):
    nc = bacc.Bacc(
        "TRN2", target_bir_lowering=False, debug=False, num_devices=N_CORES
    )

    xt_d = nc.dram_tensor("xt", [B, D, S], BF, kind="ExternalInput").ap()
    wq_d = nc.dram_tensor("wq", [D, 128], BF, kind="ExternalInput").ap()
    wk_d = nc.dram_tensor("wk", [D, 128], BF, kind="ExternalInput").ap()
    wv_d = nc.dram_tensor("wv", [D, 128], BF, kind="ExternalInput").ap()
    bq_d = nc.dram_tensor("bq", [128, 1], F32, kind="ExternalInput").ap()
    bk_d = nc.dram_tensor("bk", [128, 1], F32, kind="ExternalInput").ap()
    bv_d = nc.dram_tensor("bv", [1, 128], F32, kind="ExternalInput").ap()
    wout_d = nc.dram_tensor("wout", [D, D], BF, kind="ExternalInput").ap()
    bout_d = nc.dram_tensor("bout", [1, D], F32, kind="ExternalInput").ap()
    out_d = nc.dram_tensor("out", [512, D], F32, kind="ExternalOutput").ap()

    # per-(batch, half) resharding buffers; one AllToAll per pair of
    # q-chunks (small collectives are floor-bound ~10us on this topology,
    # so fewer/bigger wins)
    yp = {
        (b, h): nc.dram_tensor(f"yp{b}_{h}", [8, 2, 128, 64], BF)
        for b in range(B) for h in range(2)
    }
    ya = {
        (b, h): nc.dram_tensor(f"ya{b}_{h}", [8, 2, 128, 64], BF)
        for b in range(B) for h in range(2)
    }
    if dbg:
        dbg_qT = nc.dram_tensor("dbg_qT", [B, 128, S], BF, kind="ExternalOutput").ap()
        dbg_kT = nc.dram_tensor("dbg_kT", [B, 128, S], BF, kind="ExternalOutput").ap()
        dbg_v = nc.dram_tensor("dbg_v", [B, 128, N_TB * 2 * 66], BF, kind="ExternalOutput").ap()

    tri_d = nc.dram_tensor("tri", [128, 128], BF, kind="ExternalInput").ap()
    ones_d = nc.dram_tensor("vones", [128, N_TB, 2, 2], BF, kind="ExternalInput").ap()
    ones64_d = nc.dram_tensor("ones64", [1, 64], F32, kind="ExternalInput").ap()

    with tile.TileContext(nc) as tc, contextlib.ExitStack() as ctx:
        const = ctx.enter_context(tc.tile_pool(name="const", bufs=1))
        xt_pool = ctx.enter_context(tc.tile_pool(name="xtp", bufs=4))
        qk_pool = ctx.enter_context(tc.tile_pool(name="qkp", bufs=2))
        v_pool = ctx.enter_context(tc.tile_pool(name="vp", bufs=2))
        ex_pool = ctx.enter_context(tc.tile_pool(name="exp", bufs=4))
        z_pool = ctx.enter_context(tc.tile_pool(name="zp", bufs=2))
        y_pool = ctx.enter_context(tc.tile_pool(name="yp", bufs=3))
        yg_pool = ctx.enter_context(tc.tile_pool(name="ygp", bufs=2))
        o_pool = ctx.enter_context(tc.tile_pool(name="op", bufs=2))
        psum = ctx.enter_context(tc.tile_pool(name="psum", bufs=1, space="PSUM"))
        dram_pool = ctx.enter_context(tc.tile_pool(name="dramp", bufs=2, space="DRAM"))

        # ---- constant tiles ----
        wq_sb = const.tile([128, N_DC, 128], BF)
        wk_sb = const.tile([128, N_DC, 128], BF)
        wv_sb = const.tile([128, N_DC, 128], BF)
        bq_sb = const.tile([128, 1], F32)
        bk_sb = const.tile([128, 1], F32)
        bv_bc = const.tile([128, 128], F32)
        tri_sb = const.tile([128, 128], BF)
        ones64_sb = const.tile([1, 64], F32)
        warm = const.tile([128, 1], F32)

        # ---- per-batch state ----
        qTs, kTs, vsbs = {}, {}, {}

        def emit_xt_dma(b, sc, split=False):
            xt = xt_pool.tile([128, N_DC, SC], BF, tag="xt")
            src = xt_d[b].rearrange("(c p) s -> p c s", p=128)[:, :, ds(sc * SC, SC)]
            if split:
                for dc in range(N_DC):
                    nc.sync.dma_start(out=xt[:, dc, :], in_=src[:, dc, :])
            else:
                nc.sync.dma_start(out=xt, in_=src)
            return xt

        def emit_batch_tiles(b, q=None):
            q = q or nc.sync
            qT = qk_pool.tile([128, S], BF, tag="qT")
            kT = qk_pool.tile([128, S], BF, tag="kT")
            v_sb = v_pool.tile([128, N_TB, 2, 66], BF, tag="vsb")
            q.dma_start(out=v_sb[:, :, :, 64:66], in_=ones_d)
            qTs[b], kTs[b], vsbs[b] = qT, kT, v_sb

        def unit_q(b, sc, xt):
            ps = psum.tile([128, SC], F32, tag="qkv", bufs=2)
            for dc in range(N_DC):
                nc.tensor.matmul(
                    ps, wq_sb[:, dc, :], xt[:, dc, :],
                    start=(dc == 0), stop=(dc == N_DC - 1),
                )
            nc.vector.tensor_scalar_add(
                out=qTs[b][:, ds(sc * SC, SC)], in0=ps, scalar1=bq_sb
            )

        def unit_k(b, sc, xt):
            ps = psum.tile([128, SC], F32, tag="qkv", bufs=2)
            for dc in range(N_DC):
                nc.tensor.matmul(
                    ps, wk_sb[:, dc, :], xt[:, dc, :],
                    start=(dc == 0), stop=(dc == N_DC - 1),
                )
            nc.vector.tensor_scalar_add(
                out=kTs[b][:, ds(sc * SC, SC)], in0=ps, scalar1=bk_sb
            )

        def unit_v_j(b, sc, xt, j4, holder):
            # one t-sub-block of the V projection; finer grain so drained
            # units never dam up the attention pipeline
            if j4 == 0:
                holder["ps"] = psum.tile(
                    [128, SC], F32, tag="qkv", bufs=2, name="psv"
                )
            ps = holder["ps"]
            for dc in range(N_DC):
                nc.tensor.matmul(
                    ps[:, ds(j4 * 128, 128)],
                    xt[:, dc, ds(j4 * 128, 128)],
                    wv_sb[:, dc, :],
                    start=(dc == 0), stop=(dc == N_DC - 1),
                )
            nc.vector.tensor_add(
                out=vsbs[b][:, sc * 4 + j4, :, 0:64],
                in0=ps[:, ds(j4 * 128, 128)].rearrange("p (h e) -> p h e", h=2),
                in1=bv_bc.rearrange("p (h e) -> p h e", h=2),
            )

        def qkv_units(b, sc, xt):
            holder = {}
            us = [
                lambda: unit_q(b, sc, xt),
                lambda: unit_k(b, sc, xt),
            ]
            for j4 in range(4):
                us.append(lambda j4=j4: unit_v_j(b, sc, xt, j4, holder))
            return us

        # big constants, loaded in slices during the b0 q-chunk windows
        wout_sb = const.tile([128, N_DC, D], BF)
        bout_bc = const.tile([128, D], F32)
        wout_src = wout_d.rearrange("(c p) e -> p c e", p=128)

        def emit_wout_slices(lo, hi):
            for ec in range(lo, hi):
                nc.sync.dma_start(out=wout_sb[:, ec, :], in_=wout_src[:, ec, :])

        ygs = {}

        def unit_yg_load(b, rb, queues=None):
            # rows [128rb, 128rb+128) of this core's 256 tokens = chunks
            # {2rb, 2rb+1}; collectives for those chunks must be done
            queues = queues or [nc.sync]
            ygl = []
            for ec in range(8):
                yg = yg_pool.tile([128, 2, 64], BF, tag=f"yg{ec}")
                queues[ec % len(queues)].dma_start(
                    out=yg, in_=ya[(b, rb)].ap()[ec].transpose([1, 0, 2])
                )
                ygl.append(yg)
            ygs[(b, rb)] = ygl

        def unit_outproj(b, rb, ch, store_q=None, after=None):
            store_q = store_q or nc.sync
            pso = psum.tile([128, SC], F32, tag="qkv", bufs=2)
            for ec in range(8):
                mm = nc.tensor.matmul(
                    pso,
                    ygs[(b, rb)][ec].rearrange("p a i -> p (a i)"),
                    wout_sb[:, ec, ds(ch * SC, SC)],
                    start=(ec == 0), stop=(ec == 7),
                )
                if ec == 0 and after is not None:
                    # scheduler-order-only edge: keep these matmuls behind
                    # the PE warm-keeper chain in the queue
                    _add_dep(mm.ins, after.ins, sync=False,
                             reason="after warm chain")
            ot = o_pool.tile([128, SC], F32, tag="ot")
            nc.vector.tensor_add(out=ot, in0=pso, in1=bout_bc[:, ds(ch * SC, SC)])
            store_q.dma_start(
                out=out_d[ds(b * 256 + rb * 128, 128), ds(ch * SC, SC)], in_=ot
            )

        def outproj_units(b, rb):
            us = [lambda: unit_yg_load(b, rb)]
            for ch in range(2):
                us.append(lambda ch=ch: unit_outproj(b, rb, ch))
            return us

        def attn(b, qc, pending, mid=(), late=(), post=()):
            """Attention q-chunk qc for batch b; drains `pending` PE work
            units into the gaps of the exp-paced tb pipeline. `mid`/`late`
            units gate on collectives, so they only drain into the last
            8/4 slots (by when their AllToAll is certainly done — a unit
            drained too early head-of-line-blocks its whole engine queue)."""
            qT, kT, v_sb = qTs[b], kTs[b], vsbs[b]
            ntb = 4 * qc + 4
            last = b == 1 and qc == N_SC - 1
            psy = psum.tile([65, 2, SC], F32, tag="psy", bufs=1)
            mid, late = deque(mid), deque(late)
            for tb in range(ntb):
                j = tb - 4 * qc
                lo = 128 * j if j > 0 else 0
                w = SC - lo
                pss = psum.tile([128, 2, SC], F32, tag="pss", bufs=2)
                for h in range(2):
                    nc.tensor.matmul(
                        pss[:, h, ds(lo, w)],
                        kT[ds(64 * h, 64), ds(tb * 128, 128)],
                        qT[ds(64 * h, 64), ds(qc * SC + lo, w)],
                        start=True, stop=True,
                    )
                ex = ex_pool.tile([128, 2, SC], BF, tag="ex", bufs=4)
                nc.scalar.activation(
                    out=ex[:, :, ds(lo, w)], in_=pss[:, :, ds(lo, w)],
                    func=mybir.ActivationFunctionType.Exp,
                    scale=0.125,
                )
                if j >= 0:
                    for h in range(2):
                        nc.vector.tensor_mul(
                            out=ex[:, h, ds(128 * j, 128)],
                            in0=ex[:, h, ds(128 * j, 128)],
                            in1=tri_sb,
                        )
                # drain pending PE units into this slot (before the AV
                # matmuls, so they fill the PE while exp/psy-release settles)
                nleft = ntb - tb
                k = (len(pending) + nleft - 1) // nleft
                for _ in range(min(k, len(pending))):
                    pending.popleft()()
                if nleft <= 8 and mid:
                    k = (len(mid) + nleft - 1) // nleft
                    for _ in range(min(k, len(mid))):
                        mid.popleft()()
                if nleft <= 4 and late:
                    k = (len(late) + nleft - 1) // nleft
                    for _ in range(min(k, len(late))):
                        late.popleft()()
                for h in range(2):
                    nc.tensor.matmul(
                        psy[:, h, ds(lo, w)],
                        v_sb[:, tb, h, 0:65],
                        ex[:, h, ds(lo, w)],
                        start=(tb == 0), stop=(tb == ntb - 1),
                    )
            while pending:
                pending.popleft()()
            while mid:
                mid.popleft()()
            while late:
                late.popleft()()
            # normalization. The final q-chunk broadcasts the denominator
            # across partitions with a K=1 PE matmul into psy's own
            # (just-freed) rows 0-63 — no DRAM round trip, so the last
            # collective trigger fires ~AV+6us, and its stores ride the
            # (then idle) scalar queue. Earlier chunks, whose collectives
            # have many microseconds of slack, keep the cheaper DRAM
            # round-trip broadcast that stays off the PE. Custom-DVE recip
            # runs at base 0 only (it misreads psum/base!=0 operands).
            qn = nc.scalar if last else nc.sync
            if last:
                with tc.high_priority():
                    yu = z_pool.tile([64, 2, SC], F32, tag="yu", bufs=2)
                    nc.vector.tensor_copy(out=yu, in_=psy[0:64])
                    # den row copied on the (now idle) ACT engine so it
                    # overlaps the DVE's yu copy
                    d0 = z_pool.tile([1, 2 * SC], F32, tag="d0", bufs=2)
                    nc.scalar.activation(
                        out=d0,
                        in_=psy[ds(64, 1), :, :].rearrange("p a s -> p (a s)"),
                        func=mybir.ActivationFunctionType.Copy,
                    )
                    zi0 = z_pool.tile([1, 2 * SC], F32, tag="zi0", bufs=2)
                    nc.vector.reciprocal_approx_fast(out=zi0, in_=d0)
                    bcast = None
                    for h in range(2):
                        bcast = nc.tensor.matmul(
                            psy[0:64, h, :], ones64_sb,
                            zi0[:, ds(h * SC, SC)], start=True, stop=True,
                        )
                zi = psy[0:64]
            else:
                yu65 = z_pool.tile([65, 2, SC], F32, tag="yu65", bufs=2)
                nc.vector.tensor_copy(out=yu65, in_=psy)
                zd = dram_pool.tile([1, 2 * SC], F32, tag="zd", bufs=2)
                qn.dma_start(
                    out=zd, in_=yu65[ds(64, 1), :, :].rearrange("p a s -> p (a s)")
                )
                denb = z_pool.tile([64, 2 * SC], F32, tag="zb", bufs=2)
                qn.dma_start(out=denb, in_=zd.to_broadcast([64, 2 * SC]))
                ziw = z_pool.tile([64, 2, SC], F32, tag="zi", bufs=2)
                nc.vector.reciprocal_approx_fast(
                    out=ziw.rearrange("p a s -> p (a s)"), in_=denb
                )
                yu, zi = yu65, ziw
            hp = tc.high_priority() if last else contextlib.nullcontext()
            with hp:
                for h in range(2):
                    yts = y_pool.tile([64, 2, 256], BF, tag="yts", bufs=3)
                    nc.vector.tensor_mul(
                        out=yts.rearrange("p a s -> p (a s)"),
                        in0=yu[0:64, h, :], in1=zi[:, h, :],
                    )
                    qn.dma_start(
                        out=yp[(b, qc // 2)].ap()[:, qc % 2, ds(64 * h, 64), :]
                        .transpose([1, 0, 2]),
                        in_=yts.rearrange("p a s -> p (a s)").rearrange(
                            "p (j i) -> p j i", i=64
                        ),
                    )
            # post units (outproj whose data is long ready) run in the
            # collective-wait window, ordered behind the broadcast matmuls
            # so they can't delay the trigger chain
            for u in post:
                u(bcast)
            return psy

        def emit_collective(key):
            nc.gpsimd.collective_compute(
                "AllToAll",
                mybir.AluOpType.bypass,
                replica_groups=[list(range(N_CORES))],
                ins=[yp[key].ap()],
                outs=[ya[key].ap()],
            )

        # ================== main schedule ==================
        # startup-critical order: Q-unit deps first (wq, bq, x chunk 0 in
        # per-dc pieces), then the rest of the constants between them.
        xts = {}
        nc.sync.dma_start(out=wq_sb, in_=wq_d.rearrange("(c p) e -> p c e", p=128))
        nc.sync.dma_start(out=bq_sb, in_=bq_d)
        xts[(0, 0)] = emit_xt_dma(0, 0, split=True)
        nc.sync.dma_start(out=wk_sb, in_=wk_d.rearrange("(c p) e -> p c e", p=128))
        nc.sync.dma_start(out=bk_sb, in_=bk_d)
        nc.sync.dma_start(out=wv_sb, in_=wv_d.rearrange("(c p) e -> p c e", p=128))
        nc.sync.dma_start(out=bv_bc, in_=bv_d.to_broadcast([128, 128]))
        emit_batch_tiles(0)
        nc.sync.dma_start(out=tri_sb, in_=tri_d)
        nc.sync.dma_start(out=ones64_sb, in_=ones64_d)
        # preload the exp table set off the critical path
        nc.scalar.activation(
            out=warm, in_=bq_sb, func=mybir.ActivationFunctionType.Exp
        )
        xts[(0, 1)] = emit_xt_dma(0, 1, split=True)
        for u in qkv_units(0, 0, xts[(0, 0)]):
            u()

        # x prefetches are scheduled so no transfer overlaps the first
        # AllToAll's window (its duration is contention-sensitive and its
        # completion gates outproj(0,0))
        prefetch = {
            (0, 0): [(0, 2), (0, 3)],
            (0, 1): [(1, 0)],
            (0, 2): [(1, 1)],
            (1, 0): [(1, 3)],
        }
        # xt(1,2) is needed right at b1q1's start and must be fully landed
        # BEFORE the first AllToAll's window (~84-108us) or its transfer
        # both slows that collective and arrives too late for the drained
        # QKV units; its pool slot (xt(0,2)'s) is free from b0q1 on.
        prefetch_pre = {(0, 3): [(1, 2)]}
        for b in range(B):
            for qc in range(N_SC):
                for key in prefetch_pre.get((b, qc), []):
                    xts[key] = emit_xt_dma(*key)
                pending, mid, late = deque(), deque(), deque()
                if qc < 3:
                    pending.extend(qkv_units(b, qc + 1, xts[(b, qc + 1)]))
                elif b == 0:
                    # tail of b0: b1's first QKV
                    emit_batch_tiles(1)
                    pending.extend(qkv_units(1, 0, xts[(1, 0)]))
                if b == 1 and qc == 2:
                    pending.appendleft(lambda: unit_yg_load(0, 0))
                    mid.extend(
                        lambda ch=ch: unit_outproj(0, 0, ch) for ch in range(2)
                    )
                post = ()
                if b == 1 and qc == 3:
                    pending.appendleft(lambda: unit_yg_load(0, 1))
                    mid.extend(
                        lambda ch=ch: unit_outproj(0, 1, ch) for ch in range(2)
                    )
                    late.append(lambda: unit_yg_load(1, 0))
                    post = [
                        lambda a, ch=ch: unit_outproj(1, 0, ch, after=a)
                        for ch in range(2)
                    ]
                psy_last = attn(b, qc, pending, mid, late, post)
                if qc % 2 == 1:
                    emit_collective((b, qc // 2))
                # prefetch + big-constant DMAs go AFTER the normalize so
                # the sync queue serves the collective trigger path first;
                # the data is still 1+ chunks early
                for key in prefetch.get((b, qc), []):
                    xts[key] = emit_xt_dma(*key)
                if b == 0 and qc == 0:
                    emit_wout_slices(0, 4)
                elif b == 0 and qc == 1:
                    emit_wout_slices(4, 8)
                    nc.sync.dma_start(out=bout_bc, in_=bout_d.to_broadcast([128, D]))

        # tail: yg loads for the final pair ride both hwdge queues; a
        # DVE-paced junk-matmul chain keeps the PE's HAM clock warm
        # through the collective wait (a >3.4us idle gap would re-throttle
        # the PE to half clock right before the final outproj).
        unit_yg_load(1, 1, queues=[nc.sync, nc.scalar])
        ja = z_pool.tile([128, 2048], BF, tag="junk", bufs=2)
        nc.vector.tensor_copy(
            out=ja, in_=wout_sb[:, 0:2, :].rearrange("p c e -> p (c e)")
        )
        last_junk = None
        for _ in range(12):
            last_junk = nc.tensor.matmul(
                psy_last[0:64, 0, :], wq_sb[:, 0, 0:64], ja[:, 0:SC],
                start=True, stop=True,
            )
            jb = z_pool.tile([128, 2048], BF, tag="junk", bufs=2)
            nc.vector.tensor_copy(out=jb, in_=ja)
            ja = jb
        unit_outproj(1, 1, 0, store_q=nc.sync, after=last_junk)
        unit_outproj(1, 1, 1, store_q=nc.scalar)

        if dbg:
            for b in range(B):
                nc.sync.dma_start(out=dbg_qT[b], in_=qTs[b])
                nc.sync.dma_start(out=dbg_kT[b], in_=kTs[b])
                nc.sync.dma_start(
                    out=dbg_v[b], in_=vsbs[b].rearrange("p a b c -> p (a b c)")
                )

    nc.compile()
    return nc


_NC_CACHE = {}


def _get_program(dbg=False):
    if dbg not in _NC_CACHE:
        _NC_CACHE[dbg] = _build_program(dbg=dbg)
    return _NC_CACHE[dbg]


def make_in_maps(x, Wqkv, bqkv, Wout, bout):
    x = np.asarray(x, dtype=np.float32)
    Wqkv = np.asarray(Wqkv, dtype=np.float32)
    bqkv = np.asarray(bqkv, dtype=np.float32)
    Wout = np.asarray(Wout, dtype=np.float32)
    bout = np.asarray(bout, dtype=np.float32)
    bf = lambda a: np.ascontiguousarray(a).astype(ml_dtypes.bfloat16)

    xt = bf(x.transpose(0, 2, 1))  # [B, D, S]
    wout = bf(Wout)
    bout2 = np.ascontiguousarray(bout.reshape(1, D))

    tri = np.triu(np.ones((128, 128), dtype=np.float32)).astype(ml_dtypes.bfloat16)
    vones = np.ones((128, N_TB, 2, 2), dtype=np.float32).astype(ml_dtypes.bfloat16)
    in_maps = []
    for c in range(N_CORES):
        h0, h1 = 2 * c, 2 * c + 1
        wq = bf(np.concatenate([Wqkv[h0, :, 0:64], Wqkv[h1, :, 0:64]], axis=1))
        wk = bf(np.concatenate([Wqkv[h0, :, 64:128], Wqkv[h1, :, 64:128]], axis=1))
        wv = bf(np.concatenate([Wqkv[h0, :, 128:192], Wqkv[h1, :, 128:192]], axis=1))
        bq = np.ascontiguousarray(
            np.concatenate([bqkv[h0, 0:64], bqkv[h1, 0:64]]).reshape(128, 1)
        )
        bk = np.ascontiguousarray(
            np.concatenate([bqkv[h0, 64:128], bqkv[h1, 64:128]]).reshape(128, 1)
        )
        bv = np.ascontiguousarray(
            np.concatenate([bqkv[h0, 128:192], bqkv[h1, 128:192]]).reshape(1, 128)
        )
        in_maps.append(
            {
                "xt": xt, "wq": wq, "wk": wk, "wv": wv,
                "bq": bq, "bk": bk, "bv": bv,
                "wout": wout, "bout": bout2,
                "tri": tri, "vones": vones,
                "ones64": np.ones((1, 64), dtype=np.float32),
            }
        )
    return in_maps


def assemble(results):
    # core c's rows: row b*256 + qc*64 + i  ->  token qc*512 + 64c + i
    full = np.empty((B, S, D), dtype=np.float32)
    for c in range(N_CORES):
        rc = results[c]["out"].reshape(B, N_SC, 64, D)
        for b in range(B):
            for qc in range(N_SC):
                full[b, qc * 512 + 64 * c : qc * 512 + 64 * (c + 1)] = rc[b, qc]
    return full


def _install_ntff_hook():
    if "antenv.axon_hooks" in sys.modules:
        return
    so_path = "/opt/axon/libaxon_pjrt.so"
    try:
        lib = ctypes.CDLL(so_path)
        lib.axon_start_nrt_profile.argtypes = [
            ctypes.POINTER(ctypes.c_int64),
            ctypes.c_size_t,
        ]
        lib.axon_start_nrt_profile.restype = ctypes.c_int64
        lib.axon_stop_nrt_profile.argtypes = [ctypes.c_char_p]
        lib.axon_stop_nrt_profile.restype = ctypes.c_int64
    except (OSError, AttributeError):
        return

    @contextlib.contextmanager
    def _hook(output_dir, device_ids):
        import jax

        jax.devices()
        if device_ids:
            ids = (ctypes.c_int64 * len(device_ids))(*device_ids)
            rc = lib.axon_start_nrt_profile(ids, len(device_ids))
        else:
            rc = lib.axon_start_nrt_profile(None, 0)
        if rc != 0:
            raise RuntimeError(f"axon_start_nrt_profile rc={rc}")
        try:
            yield
        finally:
            n = lib.axon_stop_nrt_profile(str(output_dir).encode())
            if n < 0:
                raise RuntimeError(f"axon_stop_nrt_profile rc={n}")

    mod = types.ModuleType("antenv.axon_hooks")
    mod.get_axon_ntff_profile_hook = lambda: _hook
    mod.set_axon_ntff_profile_hook = lambda h: None
    sys.modules["antenv.axon_hooks"] = mod


def run(inputs, trace=False, dbg=False):
    from concourse.bass_utils import run_bass_kernel_spmd

    if trace:
        _install_ntff_hook()
    nc = _get_program(dbg=dbg)
    in_maps = make_in_maps(**inputs)
    res = run_bass_kernel_spmd(
        nc, in_maps, core_ids=list(range(N_CORES)), trace=trace
    )
    return assemble(res.results), res


def kernel(x, Wqkv, bqkv, Wout, bout):
    out, _ = run(
        {"x": x, "Wqkv": Wqkv, "bqkv": bqkv, "Wout": Wout, "bout": bout},
        trace=False,
    )
    return out
